# revision 13
# baseline (speedup 1.0000x reference)
"""Trainium2 Bass kernel for GQA attention (B=2, S=2048, DIM=2048, H=16, KV=8,
HD=128) with RoPE + causal mask + output projection.

Sharding: 8-way tensor parallelism over heads. Core c computes q heads
{2c, 2c+1} and kv head c end-to-end (QKV projection, RoPE, causal attention),
contributes its transposed attention output to on-device AllGathers, then
computes the output-projection column slice out[:, 256c:256(c+1)] from the
gathered activations. The host only slices inputs and concatenates outputs.

v4 changes over v3 (which ran DVE-bound in the attention phase, starving the
PE and causing HAM 1.2GHz re-throttles every chunk):
- both q heads are packed into [128, 1024] tiles through the whole softmax
  chain: one 2-bank PSUM scores tile, ONE exp activation, ONE bf16 eacc
  accumulate per j-block (vs 2 fp32 adds), halving DVE time per iteration.
- the causal mask is no longer a PE matmul adding a -30 bias: diagonal blocks
  exp first, then a cheap DVE multiply with a 0/1 triangle zeroes the masked
  region (column-sliced so only the masked prefix is touched).
- softmax denominator: ones(128,128) matmul gives the PARTITION-BROADCAST
  denominator in one shot (replaces pden + broadcast matmuls + casts), and
  1/x runs as reciprocal_approx_fast (single DVE op) instead of the 3.3us
  iterative RECIPROCAL that previously sat on the AllGather critical path.
- RoPE intermediates are bf16 so the combine add/sub runs in 2x DVE mode.
- AllGather outputs are addr_space="Shared" (fast HBM-HBM collective path).
- PE warm-up matmuls run during the initial weight DMA so the HAM clock gate
  opens before the first projection matmul.

Layout tricks:
- Everything is computed transposed (feature dim on SBUF partitions); the only
  on-device transposes are 16 PE transposes per batch for v.
- RoPE's interleaved (even, odd) pairs are handled by permuting wq/wk columns
  on the host to [evens, odds] per head, making the rotation act on two
  contiguous 64-partition halves (odd-half reads use the DVE PSUM read port's
  base-partition offset; SBUF operands stay base-0). q/k are permuted
  consistently so q.k dot products are unchanged; v / wo stay unpermuted.
- Softmax runs in scoresT layout (keys on partitions): no max subtraction
  (scores are O(5) here), no masking matmuls, denominator via ones matmul.
- Matmuls are bf16 (fp32 accumulate); 1/sqrt(HD) is folded into wq.
"""

import sys

if "/opt/trn_rl_repo" not in sys.path:
    sys.path.insert(0, "/opt/trn_rl_repo")

import numpy as np
import ml_dtypes

B, S, DIM = 2, 2048, 2048
H, KV, HD = 16, 8, 128
NC = 8
NS = B * S            # 4096 flattened (b, s) rows
P = 128
MB = DIM // P         # 16 contraction blocks for the projections
BF = ml_dtypes.bfloat16

_cache: dict = {}


def _build(debug=False):
    import concourse.bass as bass
    import concourse.mybir as mybir
    import concourse.tile as tile
    from concourse import bacc
    from concourse.masks import make_identity

    dt = mybir.dt
    f32, bf16 = dt.float32, dt.bfloat16
    Exp = mybir.ActivationFunctionType.Exp

    nc = bacc.Bacc("TRN2", debug=False, target_bir_lowering=False, num_devices=NC)

    # x^T arrives pre-tiled as [m_block, window, 128, 512] so every
    # projection-stream DMA is one contiguous 128KB block
    xT_h = nc.dram_tensor("xT", (MB, 8, P, 512), bf16, kind="ExternalInput").ap()
    # weights arrive pre-tiled as [mi=128, mb*d] so their DMAs are contiguous
    wq_h = nc.dram_tensor("wq_c", (P, MB * 256), bf16, kind="ExternalInput").ap()
    wk_h = nc.dram_tensor("wk_c", (P, MB * HD), bf16, kind="ExternalInput").ap()
    wv_h = nc.dram_tensor("wv_c", (P, MB * HD), bf16, kind="ExternalInput").ap()
    wo_h = nc.dram_tensor("wo_c", (P, MB * 256), bf16, kind="ExternalInput").ap()
    cos_h = nc.dram_tensor("cosT", (64, NS), bf16, kind="ExternalInput").ap()
    sin_h = nc.dram_tensor("sinT", (64, NS), bf16, kind="ExternalInput").ap()
    tri_h = nc.dram_tensor("tri", (P, 512), bf16, kind="ExternalInput").ap()
    out_h = nc.dram_tensor("outT", (256, NS), f32, kind="ExternalOutput").ap()
    dbg = {}
    if debug:
        for nm, shp in [("qrot_d", (P, 2 * NS)), ("krot_d", (P, NS)),
                        ("vnat_d", (P, NS)), ("oav_d", (P, 2 * NS)),
                        ("ag_d", (NC * 256, NS))]:
            dbg[nm] = nc.dram_tensor(nm, shp, bf16, kind="ExternalOutput").ap()

    with tile.TileContext(nc) as tc:
        with (
            tc.tile_pool(name="const", bufs=1) as const,
            tc.tile_pool(name="persist", bufs=1) as persist,
            tc.tile_pool(name="xs", bufs=8) as xs,
            tc.tile_pool(name="tmp", bufs=3) as tmp,
            tc.tile_pool(name="et", bufs=8) as et,
            tc.tile_pool(name="gp", bufs=8) as gp,
            tc.tile_pool(name="ot", bufs=3) as ot,
            tc.tile_pool(name="dram", bufs=1, space="DRAM") as dram,
        ):
            # ---- constants into SBUF ----
            ident = const.tile([P, P], bf16)
            make_identity(nc, ident[:])
            wq_sb = const.tile([P, MB, 256], bf16)
            nc.gpsimd.dma_start(wq_sb[:], wq_h.rearrange("p (mb d) -> p mb d", mb=MB))
            wk_sb = const.tile([P, MB, HD], bf16)
            nc.gpsimd.dma_start(wk_sb[:], wk_h.rearrange("p (mb d) -> p mb d", mb=MB))
            wv_sb = const.tile([P, MB, HD], bf16)
            nc.gpsimd.dma_start(wv_sb[:], wv_h.rearrange("p (mb d) -> p mb d", mb=MB))
            cos_sb = const.tile([64, NS], bf16)
            nc.gpsimd.dma_start(cos_sb[:], cos_h)
            sin_sb = const.tile([64, NS], bf16)
            nc.gpsimd.dma_start(sin_sb[:], sin_h)
            tri_sb = const.tile([P, 512], bf16)
            nc.gpsimd.dma_start(tri_sb[:], tri_h)
            ones128 = const.tile([P, P], bf16)
            nc.gpsimd.memset(ones128[:], 1.0)
            # wo is not needed until the first output-projection chunk; load
            # it behind everything the projection front section depends on
            wo_sb = const.tile([P, MB, 256], bf16)
            nc.gpsimd.dma_start(wo_sb[:], wo_h.rearrange("p (mb d) -> p mb d", mb=MB))

            # ---- PE warm-up during the initial weight DMA: ~6us of dummy
            # matmuls so the HAM clock gate is open when projections start
            with tc.tile_pool(name="psWu", bufs=1, space="PSUM") as psWu:
                pwu = psWu.tile([P, P], f32)
                for i in range(56):
                    nc.tensor.matmul(pwu[:], ident[:], ident[:],
                                     start=(i == 0), stop=(i == 55))

            # ---- per-batch persistent activations ----
            qrot = [persist.tile([P, 2, S], bf16, name=f"qrot{b}") for b in range(B)]
            krot = [persist.tile([P, S], bf16, name=f"krot{b}") for b in range(B)]
            vTt = [persist.tile([P, S], bf16, name=f"vTt{b}") for b in range(B)]
            vnat = [persist.tile([P, S // P, HD], bf16, name=f"vnat{b}")
                    for b in range(B)]
            ag_in = [[dram.tile([256, 512], bf16, name=f"agi{b}{t}")
                      for t in range(4)] for b in range(B)]
            ag_out = [[dram.tile([NC * 256, 512], bf16, addr_space="Shared",
                                 name=f"ago{b}{t}")
                       for t in range(4)] for b in range(B)]

            last_xt = [None, None]
            from concourse.tile_rust import add_dep_helper

            def rope_unit(src, cos_c, sin_c, out_even, out_odd):
                # ACT first copies the two PSUM halves to bf16 SBUF: this
                # releases the projection accumulator almost immediately (so
                # the PSUM pools can single-buffer) and lets every DVE op run
                # in 2x packed-bf16 mode (327ns vs 658ns for a PSUM-source op)
                ev = tmp.tile([64, 512], bf16, tag="rpe", name="ev")
                od = tmp.tile([64, 512], bf16, tag="rpo", name="od")
                nc.scalar.copy(ev[:], src[0:64, :])
                nc.scalar.copy(od[:], src[64:128, :])
                t1 = tmp.tile([64, 512], bf16, tag="r1", name="r1")
                t2 = tmp.tile([64, 512], bf16, tag="r2", name="r2")
                nc.vector.tensor_mul(t1[:], ev[:], cos_c)
                nc.vector.tensor_mul(t2[:], od[:], sin_c)
                nc.vector.tensor_sub(out_even, t1[:], t2[:])
                t3 = tmp.tile([64, 512], bf16, tag="r1", name="r3")
                t4 = tmp.tile([64, 512], bf16, tag="r2", name="r4")
                nc.vector.tensor_mul(t3[:], ev[:], sin_c)
                nc.vector.tensor_mul(t4[:], od[:], cos_c)
                nc.vector.tensor_add(out_odd, t3[:], t4[:])

            def transpose_v(b, blk, pool):
                # one v-block PE transpose, interleaved into the projection
                # or attention stream so the 16-transpose chain never sits on
                # the critical path between projections and attention
                pt = pool.tile([P, P], bf16, tag=pool.transpose_tag, bufs=None,
                               name="pt")
                nc.tensor.transpose(
                    pt[:], vTt[b][:, blk * P:(blk + 1) * P], ident[:])
                nc.scalar.copy(vnat[b][:, blk, :], pt[:])

            for b in range(B):
                # ---- projections (transposed layout) + RoPE; window sp's v
                # transposes ride inside window sp+1's matmul stream.  All
                # accumulators single-buffer: the ACT copies in rope_unit /
                # vTt release them within ~1us of the window's last matmul ----
                with tc.tile_pool(name=f"psA{b}", bufs=1, space="PSUM") as psA:
                    psA.transpose_tag = "pt"
                    for sp in range(4):          # 512-col windows within batch
                        gw = slice(b * S + sp * 512, b * S + (sp + 1) * 512)
                        lw = slice(sp * 512, (sp + 1) * 512)
                        pq = [psA.tile([P, 512], f32, tag=f"pq{h}", name=f"pq{h}")
                              for h in range(2)]
                        pk = psA.tile([P, 512], f32, tag="pk", name="pk")
                        pv = psA.tile([P, 512], f32, tag="pv", name="pv")
                        for m in range(MB):
                            xt = xs.tile([P, 512], bf16, tag="xt", name="xt")
                            last_xt[b] = nc.sync.dma_start(
                                xt[:], xT_h[m, b * 4 + sp])
                            for acc, lhsT in (
                                (pq[0], wq_sb[:, m, 0:128]),
                                (pq[1], wq_sb[:, m, 128:256]),
                                (pk, wk_sb[:, m, :]),
                                (pv, wv_sb[:, m, :]),
                            ):
                                nc.tensor.matmul(
                                    acc[:], lhsT, xt[:],
                                    start=(m == 0), stop=(m == MB - 1),
                                )
                            if sp > 0 and m % 4 == 3:
                                transpose_v(b, (sp - 1) * 4 + m // 4, psA)
                        cos_c, sin_c = cos_sb[:, gw], sin_sb[:, gw]
                        for h in range(2):
                            rope_unit(pq[h], cos_c, sin_c,
                                      qrot[b][0:64, h, lw], qrot[b][64:128, h, lw])
                        rope_unit(pk, cos_c, sin_c,
                                  krot[b][0:64, lw], krot[b][64:128, lw])
                        nc.scalar.copy(vTt[b][:, lw], pv[:])

                # ---- causal attention in scoresT layout, both heads packed
                # into [128, 1024] tiles (head h occupies cols 512h:512h+512).
                # For b==1, the previous batch's output-projection chunks are
                # interleaved between attention chunks as PE filler, and this
                # batch's output projection runs in the same pool context so
                # nothing serializes behind a closed phase.
                with (
                    tc.tile_pool(name=f"psS{b}", bufs=2, space="PSUM") as psS,
                    tc.tile_pool(name=f"psV{b}", bufs=2, space="PSUM") as psV,
                ):
                    psS.transpose_tag = "ps"

                    def finalize(fin):
                        """Normalize a finished chunk and fire its AllGather.
                        Runs at the top of the next chunk: the denominator
                        matmuls go straight onto the PE stream while the DVE
                        reciprocal+scale overlap the next chunk's j-loop."""
                        pav_f, eacc_f, t_f = fin
                        den = psS.tile([P, 1024], f32, tag="ps", name="den")
                        for h in range(2):
                            hs = slice(512 * h, 512 * h + 512)
                            nc.tensor.matmul(den[:, hs], ones128[:],
                                             eacc_f[:, hs], start=True, stop=True)
                        rcp = tmp.tile([P, 1024], f32, tag="rcp", name="rcp")
                        nc.vector.reciprocal_approx_fast(out=rcp[:], in_=den[:])
                        oavt = tmp.tile([P, 1024], bf16, tag="oav", name="oavt")
                        nc.vector.tensor_mul(oavt[:], pav_f[:], rcp[:])
                        for h in range(2):
                            nc.gpsimd.dma_start(
                                ag_in[b][t_f][h * P:(h + 1) * P, :],
                                oavt[:, 512 * h:512 * h + 512],
                            )
                        nc.gpsimd.collective_compute(
                            "AllGather",
                            mybir.AluOpType.bypass,
                            replica_groups=[list(range(NC))],
                            ins=[ag_in[b][t_f].opt()],
                            outs=[ag_out[b][t_f].opt()],
                        )
                        return oavt

                    def wo_chunk(bb, tt):
                        """out[:, 256c:256c+256] slice for gathered chunk
                        (bb, tt): 16 row-block loads + 32 matmuls, PSUM from
                        the pav rotation."""
                        pw = psV.tile([P, 1024], f32, tag="pav", name="pw")
                        for r in range(MB):
                            g = gp.tile([P, 512], bf16, tag="g", name="g")
                            gl = nc.sync.dma_start(
                                g[:], ag_out[bb][tt][r * P:(r + 1) * P, :])
                            # keep gathered-chunk loads behind the xt stream on
                            # the in-order SP queue: a load waiting on its
                            # AllGather must never starve projection loads
                            add_dep_helper(
                                gl.ins, last_xt[B - 1].ins, sync=False,
                                reason="wo chunk loads after activation stream",
                            )
                            for n in range(2):
                                nc.tensor.matmul(
                                    pw[:, 512 * n:512 * n + 512],
                                    wo_sb[:, r, n * 128:(n + 1) * 128],
                                    g[:], start=(r == 0), stop=(r == MB - 1),
                                )
                        for n in range(2):
                            o = ot.tile([P, 512], f32, tag="o", name="o")
                            nc.scalar.copy(o[:], pw[:, 512 * n:512 * n + 512])
                            nc.sync.dma_start(
                                out_h[n * P:(n + 1) * P,
                                      bb * S + tt * 512: bb * S + (tt + 1) * 512],
                                o[:],
                            )

                    pending = None
                    for t in range(4):            # query chunks of 512
                        if pending is not None:
                            finalize(pending)
                            pending = None
                        if b == 1:
                            wo_chunk(0, t)
                            if t == 3:
                                # (1,0)'s AllGather has landed by now; its
                                # projection overlaps the longest attn chunk
                                wo_chunk(1, 0)
                        il = slice(t * 512, (t + 1) * 512)
                        pav = psV.tile([P, 1024], f32, tag="pav", name="pav")
                        eacc = tmp.tile([P, 1024], bf16, tag="eacc", name="eacc")
                        nj = 4 * t + 4
                        # descending j: the masked diagonal blocks (which add a
                        # DVE dependency to e) run first, so the pipeline drain
                        # at the end only waits on plain exps
                        order = list(range(nj - 1, -1, -1))
                        pipe = []
                        for idx, j in enumerate(order):
                            rel = j - 4 * t
                            ps = psS.tile([P, 1024], f32, tag="ps", name="ps")
                            for h in range(2):
                                nc.tensor.matmul(
                                    ps[:, 512 * h:512 * h + 512],
                                    krot[b][:, j * P:(j + 1) * P],
                                    qrot[b][:, h, il], start=True, stop=True,
                                )
                            e = et.tile([P, 1024], bf16, tag="e", name="e")
                            nc.scalar.activation(e[:], ps[:], Exp)
                            if t == 0:
                                # last projection window's v transposes ride
                                # here (their vnat blocks are first needed by
                                # chunk 3), keeping the proj->attention
                                # transition free of the serial chain
                                transpose_v(b, 12 + idx, psS)
                            if rel >= 0:
                                # zero the causally-masked prefix: cols
                                # [0, 128(rel+1)) against the shifted triangle
                                w = 128 * (rel + 1)
                                for h in range(2):
                                    nc.vector.tensor_mul(
                                        e[:, 512 * h:512 * h + w],
                                        e[:, 512 * h:512 * h + w],
                                        tri_sb[:, 384 - 128 * rel:
                                               384 - 128 * rel + w],
                                    )
                            if idx == 0:
                                nc.vector.tensor_copy(eacc[:], e[:])
                            else:
                                nc.vector.tensor_add(eacc[:], eacc[:], e[:])
                            pipe.append((e, idx, j))
                            if len(pipe) > 3:
                                ep, ip, jp = pipe.pop(0)
                                for h in range(2):
                                    nc.tensor.matmul(
                                        pav[:, 512 * h:512 * h + 512],
                                        vnat[b][:, jp, :],
                                        ep[:, 512 * h:512 * h + 512],
                                        start=(ip == 0), stop=(ip == nj - 1),
                                    )
                        for ep, ip, jp in pipe:
                            for h in range(2):
                                nc.tensor.matmul(
                                    pav[:, 512 * h:512 * h + 512],
                                    vnat[b][:, jp, :],
                                    ep[:, 512 * h:512 * h + 512],
                                    start=(ip == 0), stop=(ip == nj - 1),
                                )
                        pending = (pav, eacc, t)
                    finalize(pending)
                    pending = None
                    if b == 1:
                        for t in range(1, 4):
                            wo_chunk(1, t)

            if debug:
                for b in range(B):
                    for h in range(2):
                        nc.sync.dma_start(
                            dbg["qrot_d"][:, h * NS + b * S: h * NS + (b + 1) * S],
                            qrot[b][:, h, :])
                    nc.sync.dma_start(dbg["krot_d"][:, b * S:(b + 1) * S], krot[b][:])
                    nc.sync.dma_start(
                        dbg["vnat_d"].rearrange("p (bb d) -> p bb d", bb=NS // P)
                        [:, b * (S // P):(b + 1) * (S // P), :], vnat[b][:])
                    for t in range(4):
                        nc.sync.dma_start(
                            dbg["ag_d"][:, b * S + t * 512: b * S + (t + 1) * 512],
                            ag_out[b][t][:])

    nc.compile()
    return nc


def _prep_inputs(x, freqs_cos, freqs_sin, wq, wk, wv, wo):
    x = np.asarray(x, np.float32).reshape(NS, DIM)
    xT = np.ascontiguousarray(
        x.T.reshape(MB, P, 8, 512).transpose(0, 2, 1, 3)).astype(BF)
    cos = np.asarray(freqs_cos, np.float32)
    sin = np.asarray(freqs_sin, np.float32)
    cosT = np.ascontiguousarray(np.tile(cos, (B, 1)).T).astype(BF)
    sinT = np.ascontiguousarray(np.tile(sin, (B, 1)).T).astype(BF)

    perm = np.r_[np.arange(0, HD, 2), np.arange(1, HD, 2)]
    scale = np.float32(1.0 / np.sqrt(HD))
    wq = np.asarray(wq, np.float32) * scale
    wk = np.asarray(wk, np.float32)
    wv = np.asarray(wv, np.float32)
    wo = np.asarray(wo, np.float32)

    # 0/1 keep-mask for the rel=3 diagonal block; rel=r reads the slice
    # shifted left by 128(3-r).  tri[p, c] = 0 iff c < 384 + p.
    cc, pp = np.meshgrid(np.arange(512), np.arange(P))
    tri = np.ascontiguousarray((cc >= pp + 384).astype(np.float32)).astype(BF)

    def tile_w(w):
        # (2048, d) -> (128, 16*d): row mi holds [mb, d] contiguously
        d = w.shape[1]
        return np.ascontiguousarray(
            w.reshape(MB, P, d).transpose(1, 0, 2).reshape(P, MB * d)).astype(BF)

    in_maps = []
    for c in range(NC):
        wq_c = wq[:, c * 256:(c + 1) * 256]
        wq_cp = np.concatenate([wq_c[:, h * HD + perm] for h in range(2)], axis=1)
        in_maps.append({
            "xT": xT,
            "wq_c": tile_w(wq_cp),
            "wk_c": tile_w(wk[:, c * HD:(c + 1) * HD][:, perm]),
            "wv_c": tile_w(wv[:, c * HD:(c + 1) * HD]),
            "wo_c": tile_w(wo[:, c * 256:(c + 1) * 256]),
            "cosT": cosT,
            "sinT": sinT,
            "tri": tri,
        })
    return in_maps


def _run(inputs, trace=False, **kw):
    from concourse.bass_utils import run_bass_kernel_spmd

    if "nc" not in _cache:
        _cache["nc"] = _build()
    nc = _cache["nc"]
    in_maps = _prep_inputs(**inputs)
    res = run_bass_kernel_spmd(
        nc, in_maps, core_ids=list(range(NC)), trace=trace, **kw
    )
    out = np.empty((NS, DIM), np.float32)
    for c in range(NC):
        out[:, c * 256:(c + 1) * 256] = res.results[c]["outT"].T
    return out.reshape(B, S, DIM), res


def kernel(**inputs) -> np.ndarray:
    out, _ = _run(inputs, trace=False)
    return out
